# revision 20
# baseline (speedup 1.0000x reference)
"""Multi-head dense attention (no softmax) on 8 Trainium2 NeuronCores.

Math (per batch b, head h with head_dim d=64):
    q   = x @ W^T                      # [S, H] projection
    out_h = (q_h x_h^T) x_h            # naive: O(S^2 d) with an SxS temp
          = q_h (x_h^T x_h)            # reassociated: Gram matrix G_h [d, d]
The reassociation is exact (same sum, different order) and collapses the
FLOPs ~5x while removing the SxS intermediate entirely.

Sharding: core c handles batch b = c//2 and head-group hg = c%2 (8 heads,
512 output columns). Cores are fully independent (no collectives).

Device layout per core (all inputs fp16; W is pre-scaled by 1024 on the
host so its sigma~9e-5 entries clear fp16's subnormal cutoff; the Gram
tile copy multiplies by 1/1024 to undo it):
    xT  [1024, 2048]  x[b] transposed (host-prepped)  - projection operands
    xn  [2048, 512]   x[b] natural, this head-group's columns - Gram operands
    wT  [1024, 512]   1024 * W rows of this head-group, transposed (k-major)
    outT [512, 2048]  output transposed (fp16); host transposes/upcasts back

Schedule notes (v2). The kernel is PE-issue-bound: the projection alone
streams 64K rhs columns (~27.6us warm). The schedule therefore aims at
near-100% PE density from first-byte arrival:
  - ~30 junk warmup matmuls on a zeroed tile run during the DMA spin-up
    so the PE HAM clock-gate (4/8 cold -> 8/8 warm after ~3.4us of
    activity) is already warm when real work starts.
  - DMA stream order front-loads xT (kt7 lands ~5MB in) and streams xn
    last; the Gram chunks are issued interleaved into the halfB
    instruction stream at the positions where their xn chunks land.
  - halfA = k-tiles 0-2 (paced by the early stream), halfB = k-tiles 3-7
    with the PSUM->SBUF drain folding qA in (qT = psum + qA).
  - Output stage per m-tile: outT_p = Gbd_p^T qT_p, one N=512 matmul per
    s-chunk; outs for mt 0/1 are issued right after the Gram finalize so
    only mt 2/3's outs trail the last projection chain.
  - Drains alternate vector/scalar/gpsimd engines so no single engine's
    copy queue gates the PE at chain boundaries; all stores ride the
    sync-engine HWDGE ring (the baseline's gpsimd SWDGE stores were
    descriptor-bound and dragged the tail out ~7us).
PSUM: 4 banks proj chains + 1 bank (4 packed 128-col Gram chains) +
2 banks output stage = 14KB/partition of 16KB.
"""

import numpy as np

B, S, H = 4, 2048, 1024
N_HEADS = 16
HD = H // N_HEADS  # 64
N_CORES = 8
MG = H // 2        # 512 output columns per core
P = 128
KT = H // P        # 8 k-tiles
ST = S // P        # 16 s-tiles
MT = MG // P       # 4 m-tiles == head pairs
SC = S // 512      # 4 s-chunks
W_SCALE = 1024.0
KT_A = 3           # k-tiles in the first projection half
N_WARM = 30

_NC_CACHE = {}


def _build_nc():
    import concourse.mybir as mybir
    from concourse import bacc
    from concourse.tile import TileContext

    f32 = mybir.dt.float32
    f16 = mybir.dt.float16

    nc = bacc.Bacc()
    xT_d = nc.declare_dram_parameter("xT", [H, S], f16, isOutput=False)
    xn_d = nc.declare_dram_parameter("xn", [S, MG], f16, isOutput=False)
    wT_d = nc.declare_dram_parameter("wT", [MT * P, KT * P], f16, isOutput=False)
    outT_d = nc.declare_dram_parameter("outT", [MG, S], f16, isOutput=True)

    xT_t = xT_d.rearrange("(kt p) s -> p kt s", p=P)   # [128, 8, 2048]
    xn_t = xn_d.rearrange("(st p) m -> p st m", p=P)   # [128, 16, 512]
    wT_t = wT_d.rearrange("(mt p) (kt m) -> p mt kt m", p=P, m=P)  # [128, 4, 8, 128]

    with TileContext(nc) as tc:
        with (
            tc.tile_pool(name="big", bufs=1) as big,
            tc.tile_pool(name="gp", bufs=1) as gpool,
            tc.tile_pool(name="stage", bufs=4) as stage,
            tc.tile_pool(name="ps", bufs=1, space="PSUM") as ps,
        ):
            xT_sb = big.tile([P, KT, S], f16, tag="xT")
            xn_sb = big.tile([P, ST, MG], f16, tag="xn")
            wT_sb = big.tile([P, MT, KT, P], f16, tag="wT")
            qT_sb = big.tile([P, MT, S], f16, tag="qT")
            warm_sb = big.tile([P, P], f16, tag="warm")

            nc.gpsimd.memset(warm_sb, 0.0)

            # ---- DMA stream, single SP-engine HWDGE ring (the sync
            # engine is otherwise idle early, so its ~0.6us-per-
            # instruction issue cost never delays the drain copies,
            # which live on scalar/vector), in consumption order. xT is
            # front-loaded (kt7 lands ~5MB in); xn streams last, paced
            # against the Gram chunks injected into the mt2 block.
            nc.sync.dma_start(out=wT_sb[:, 0], in_=wT_t[:, 0])
            nc.sync.dma_start(out=xT_sb[:, 0, :1024], in_=xT_t[:, 0, :1024])
            nc.sync.dma_start(out=xT_sb[:, 0, 1024:], in_=xT_t[:, 0, 1024:])
            nc.sync.dma_start(out=wT_sb[:, 1], in_=wT_t[:, 1])
            nc.sync.dma_start(out=xT_sb[:, 1], in_=xT_t[:, 1])
            nc.sync.dma_start(out=xT_sb[:, 2], in_=xT_t[:, 2])
            nc.sync.dma_start(out=xT_sb[:, 3], in_=xT_t[:, 3])
            nc.sync.dma_start(out=wT_sb[:, 2:4], in_=wT_t[:, 2:4])
            for kt in range(4, KT):
                nc.sync.dma_start(out=xT_sb[:, kt], in_=xT_t[:, kt])
            for j in range(8):
                nc.sync.dma_start(
                    out=xn_sb[:, 2 * j:2 * j + 2], in_=xn_t[:, 2 * j:2 * j + 2]
                )

            # All 8 PSUM banks live in one pool, tags b0-b7. Phase 1
            # (mt0/mt1) holds all eight as open projection chains; the
            # same slots are then recycled for the mt2/mt3 chains, the
            # four Gram chains (a chain's start=True clears has_written
            # for its whole bank, so each needs a bank to itself) and
            # the output-stage matmuls.
            def bank(i, name):
                return ps.tile([P, 512], f32, tag=f"b{i}", name=name)

            # ---- PE warmup on the zeroed tile (results discarded).
            psw = bank(4, "psw")
            for i in range(N_WARM):
                nc.tensor.matmul(
                    psw[:, 0:P], lhsT=warm_sb, rhs=warm_sb, start=True, stop=True
                )

            def filler(n):
                # Junk matmuls with no data deps: they run during
                # DMA-paced waits and keep the PE HAM activity window
                # busy (idle gaps re-throttle the PE clock to 1.2 GHz).
                for _ in range(n):
                    nc.tensor.matmul(
                        psw[:, 0:P], lhsT=warm_sb, rhs=warm_sb,
                        start=True, stop=True,
                    )

            # ---- Phase 1: mt0/mt1 projection chains over all 8
            # k-tiles. mt0's first three k-tile groups go first, with
            # fillers bridging the DMA-paced waits (fillers write bank 4,
            # so they are only legal before mt1's chains open); mt1 then
            # catches up on the resident k-tiles and the rest runs
            # kt-outer as tiles land.
            p1 = {}

            def p1_mm(mt, n, sc):
                nc.tensor.matmul(
                    p1[(mt, sc)],
                    lhsT=wT_sb[:, mt, n],
                    rhs=xT_sb[:, n, sc * 512:(sc + 1) * 512],
                    start=(n == 0),
                    stop=(n == KT - 1),
                )

            for sc in range(SC):
                p1[(0, sc)] = bank(sc, f"psq0_{sc}")
            for n in range(3):
                for sc in range(SC):
                    p1_mm(0, n, sc)
                    if n == 0 and sc == 1:
                        filler(6)
                filler(4 if n == 0 else 3)
            for sc in range(SC):
                p1[(1, sc)] = bank(4 + sc, f"psq1_{sc}")
            for n in range(3):
                for sc in range(SC):
                    p1_mm(1, n, sc)
            for n in range(3, KT):
                for mt in range(2):
                    for sc in range(SC):
                        p1_mm(mt, n, sc)
            # Drain order puts banks 0 and 4 first so the mt2 chains and
            # the first Gram chunk can start immediately.
            for sc in range(SC):
                for mt in range(2):
                    sl = slice(sc * 512, (sc + 1) * 512)
                    eng = nc.vector if mt == 0 else nc.scalar
                    if mt == 0:
                        nc.vector.tensor_copy(out=qT_sb[:, mt, sl], in_=p1[(mt, sc)])
                    else:
                        nc.scalar.copy(out=qT_sb[:, mt, sl], in_=p1[(mt, sc)])

            # ---- Gram chains: pair p in bank 4+p, columns 0:128.
            gram_ps = {}

            def gram_chunk(j):
                for st in (2 * j, 2 * j + 1):
                    for p_i in range(MT):
                        if st == 0:
                            gram_ps[p_i] = bank(4 + p_i, f"psg{p_i}")
                        xp = xn_sb[:, st, p_i * P:(p_i + 1) * P]
                        nc.tensor.matmul(
                            gram_ps[p_i][:, 0:P],
                            lhsT=xp,
                            rhs=xp,
                            start=(st == 0),
                            stop=(st == ST - 1),
                        )

            gbd = {}

            def gram_finalize():
                for p_i in range(MT):
                    g = gpool.tile([P, P], f16, tag=f"g{p_i}", name=f"g{p_i}")
                    pslice = gram_ps[p_i][:, 0:P]
                    nc.gpsimd.memset(g, 0.0)
                    eng = nc.vector if p_i % 2 == 0 else nc.scalar
                    if p_i % 2 == 0:
                        nc.vector.tensor_scalar_mul(
                            out=g[0:HD, 0:HD], in0=pslice[0:HD, 0:HD],
                            scalar1=1.0 / W_SCALE,
                        )
                        nc.vector.tensor_scalar_mul(
                            out=g[HD:P, HD:P], in0=pslice[HD:P, HD:P],
                            scalar1=1.0 / W_SCALE,
                        )
                    else:
                        nc.scalar.mul(g[0:HD, 0:HD], pslice[0:HD, 0:HD],
                                      1.0 / W_SCALE)
                        nc.scalar.mul(g[HD:P, HD:P], pslice[HD:P, HD:P],
                                      1.0 / W_SCALE)
                    gbd[p_i] = g

            # ---- Output stage: out psums rotate through banks 4-7
            # (freed by the Gram finalize); casts and stores alternate
            # engines so no queue serializes the tail.
            out_n = [0]

            def out_one(p_i, sc, cast_eng):
                sl = slice(sc * 512, (sc + 1) * 512)
                pso = bank(4 + out_n[0] % 4, f"pso{p_i}_{sc}")
                out_n[0] += 1
                nc.tensor.matmul(
                    pso, lhsT=gbd[p_i], rhs=qT_sb[:, p_i, sl],
                    start=True, stop=True,
                )
                ot = stage.tile(
                    [P, 512], f16, tag=f"ot{sc % 2}", name=f"ot{p_i}_{sc}"
                )
                if cast_eng == "v":
                    nc.vector.tensor_copy(out=ot, in_=pso)
                else:
                    nc.scalar.copy(out=ot, in_=pso)
                return ot

            def store(p_i, sc, ot, eng):
                sl = slice(sc * 512, (sc + 1) * 512)
                eng.dma_start(out=outT_d[p_i * P:(p_i + 1) * P, sl], in_=ot)

            def emit_out(p_i):
                ots = []
                for sc in range(SC):
                    ots.append(out_one(p_i, sc, "v" if sc % 2 == 0 else "s"))
                for sc in range(SC):
                    store(p_i, sc, ots[sc], nc.sync if sc % 2 == 0 else nc.scalar)

            # ---- Phase 2: mt2 chains (banks 0-3) interleaved with the
            # Gram chunks (banks 4-7, paced by the late xn stream), then
            # outs for mt0/1, then mt3 with a per-s-chunk
            # drain -> out -> store pipeline so only one s-chunk trails.
            p2 = {}
            for sc in range(SC):
                p2[sc] = bank(sc, f"psq2_{sc}")
            for n in range(KT):
                for sc in range(SC):
                    nc.tensor.matmul(
                        p2[sc],
                        lhsT=wT_sb[:, 2, n],
                        rhs=xT_sb[:, n, sc * 512:(sc + 1) * 512],
                        start=(n == 0),
                        stop=(n == KT - 1),
                    )
                if n % 2 == 1:
                    gram_chunk(n // 2)
            for sc in range(SC):
                sl = slice(sc * 512, (sc + 1) * 512)
                if sc % 2 == 0:
                    nc.vector.tensor_copy(out=qT_sb[:, 2, sl], in_=p2[sc])
                else:
                    nc.scalar.copy(out=qT_sb[:, 2, sl], in_=p2[sc])
            for j in range(4, 8):
                gram_chunk(j)
            gram_finalize()
            emit_out(0)
            emit_out(1)

            # mt3 runs sc-outer on resident data with a 1-deep software
            # pipeline: chain(sc) -> [out2 piece] -> qT drain(sc) ->
            # out3(sc) matmul -> drain -> store, with drains alternating
            # vector/scalar, so only the final s-chunk's drain + 128KB
            # store trail the last matmul.
            def qdrain3(sc):
                sl = slice(sc * 512, (sc + 1) * 512)
                if sc % 2 == 0:
                    nc.vector.tensor_copy(out=qT_sb[:, 3, sl], in_=p3[sc])
                else:
                    nc.scalar.copy(out=qT_sb[:, 3, sl], in_=p3[sc])

            def out3(sc):
                ot = out_one(3, sc, "s" if sc % 2 == 0 else "v")
                store(3, sc, ot, nc.sync if sc % 2 == 0 else nc.scalar)

            p3 = {}
            for sc in range(SC):
                p3[sc] = bank(sc, f"psq3_{sc}")
            for n in range(KT):
                for sc in range(SC):
                    nc.tensor.matmul(
                        p3[sc],
                        lhsT=wT_sb[:, 3, n],
                        rhs=xT_sb[:, n, sc * 512:(sc + 1) * 512],
                        start=(n == 0),
                        stop=(n == KT - 1),
                    )
                if n == 3:
                    emit_out(2)
            for sc in range(SC):
                qdrain3(sc)
            for sc in range(SC):
                out3(sc)
    nc.compile()
    return nc


def _get_nc():
    if "nc" not in _NC_CACHE:
        _NC_CACHE["nc"] = _build_nc()
    return _NC_CACHE["nc"]


def make_in_maps(hidden_states, queries_weight):
    hs = np.ascontiguousarray(np.asarray(hidden_states, dtype=np.float32))
    w = np.ascontiguousarray(np.asarray(queries_weight, dtype=np.float32))
    in_maps = []
    for c in range(N_CORES):
        b, hg = divmod(c, 2)
        xb = hs[b]
        in_maps.append({
            "xT": np.ascontiguousarray(xb.T).astype(np.float16),
            "xn": np.ascontiguousarray(xb[:, hg * MG:(hg + 1) * MG]).astype(
                np.float16
            ),
            "wT": np.ascontiguousarray(
                (w[hg * MG:(hg + 1) * MG, :].T * W_SCALE)
                .reshape(KT, P, MT, P)
                .transpose(2, 1, 0, 3)
                .reshape(MT * P, KT * P)
            ).astype(np.float16),
        })
    return in_maps


def assemble_output(results):
    out = np.empty((B, S, H), dtype=np.float32)
    for c in range(N_CORES):
        b, hg = divmod(c, 2)
        out[b, :, hg * MG:(hg + 1) * MG] = results[c]["outT"].T.astype(np.float32)
    return out


def kernel(hidden_states, queries_weight):
    from concourse.bass_utils import run_bass_kernel_spmd

    in_maps = make_in_maps(hidden_states, queries_weight)
    res = run_bass_kernel_spmd(
        _get_nc(), in_maps, core_ids=list(range(N_CORES))
    ).results
    return assemble_output(res)


if __name__ == "__main__":
    x = np.random.randn(B, S, H).astype(np.float32)
    w = np.random.randn(H, H).astype(np.float32) * 1e-4
    out = kernel(x, w)
    print(out.shape, out.dtype)
